# revision 34
# baseline (speedup 1.0000x reference)
"""Trainium (trn2) Bass kernel for a 2-layer GAT over N=100k nodes / E=1.7M edges.

Strategy (v2 — gather-streamed edge phase)
------------------------------------------
Edges are sorted by destination on the host (index-only preprocessing); the
destination axis is sharded across the 8 NeuronCores in contiguous 128-node
windows (98 per core).  Three SPMD kernels per forward pass:

* N1 (node phase): H1ext = x @ [W1 | W1.a_src | W1.a_dst]  -> [N, 144] f16
  table, node windows sharded across cores.
* host (permutation/cast only, no FLOPs): gather H1ext rows by edge source
  (h + a_src.h) and by edge destination (a_dst.h), pack them together with
  the relative-destination column into a DMA-friendly per-core stream laid
  out [128 partitions][T tiles, C cols] so every partition reads long
  contiguous runs.
* E1 (edge phase L1): per 128-edge tile: z = als+ald; Prelu; one batched
  Exp expanded to all 128 message columns (so the message multiply is an
  all-SBUF packed-f16 TensorTensor in 2x DVE mode); the one-hot
  S[e,n]=(dst_rel==n) arrives as a host-built fp8 stream (exact for 0/1)
  and feeds the PE directly as the stationary of ONE fp8xf16 matmul per
  tile that accumulates both the segment sum and the softmax denominators
  (exp rides as 8 extra message columns).  The per-window epilogue divides
  by the denominator, applies ELU, and fuses layer 2's node matmul (PE
  transpose + o2 @ [W2 | W2.a_src2 | W2.a_dst2]) so E2 only needs
  66-column gathers.
* E2 (edge phase L2): same structure with 1 head / 64 channels; outputs the
  final [N, 64] f32.

All floating-point work runs on device; the host only sorts/gathers/casts.
The NEFF is specialized to the edge distribution and cached.

Environment workarounds: this walrus build allows only ONE semaphore wait
per instruction (split onto nop carriers post-scheduling), and the GPSIMD
ucode libraries are absent (no dma_gather/indirect-DMA fast paths - hence
the host-gathered streams).
"""
import sys
import os
import time

import numpy as np

import concourse.bass as bass
import concourse.mybir as mybir
import concourse.tile as tile
from concourse.bass_utils import run_bass_kernel_spmd

P = 128
F16 = mybir.dt.float16
F32 = mybir.dt.float32
F8 = mybir.dt.float8e4
AF = mybir.ActivationFunctionType
OP = mybir.AluOpType
NEG_SLOPE = 0.2
EXP_BIAS = -4.0     # exp(z + EXP_BIAS): constant shift cancels in softmax
GRP = 32            # tiles per stream group
PAD_REL = 255.0     # rel value for pad slots -> is_equal never matches
N_CORES = 8

# engine-assignment tuning knobs (read at kernel-build time)
# NOTE: Pool/GPSIMD offload measured ~10x slower on real HW than the
# TimelineSim cost model predicts (software Q7 ucode) - keep everything off
# the Pool engine.
CFG = {
    "pool_s_num": 0,      # pool_s_num of every pool_s_den S-builds on Pool
    "pool_s_den": 8,
    "z_add_pool": False,  # z = als+ald on Pool instead of DVE
    "epi_pool": False,    # ELU min/max on Pool
    "copy_act": True,     # epilogue PSUM->SBUF copies on ACT (Copy act)
    "grp": 32,            # tiles per stream group
    # exp expansion path per group: exp_act_num of every exp_act_den groups
    # use ACT-expanded exp + packed DVE multiply; the rest multiply against
    # a broadcast exp AP directly on DVE (slower per element, no ACT cost)
    "exp_act_num": 8,
    "exp_act_den": 8,
}

# ------------------------------------------------------------------ patches

_wsplit_counter = [0]


def _split_excess_waits(nc, max_waits=1):
    """This walrus build rejects >1 sem-wait per instruction ("Too many sync
    wait commands"). Move overflow waits onto same-engine nop carriers."""
    n_split = 0
    for f in nc.m.functions:
        for blk in f.blocks:
            changed = False
            out = []
            for inst in blk.instructions:
                si = inst.sync_info
                if si is not None and len(si.on_wait) > max_waits:
                    waits = list(si.on_wait)
                    keep = waits[len(waits) - max_waits:]
                    overflow = waits[: len(waits) - max_waits]
                    for i in range(0, len(overflow), max_waits):
                        _wsplit_counter[0] += 1
                        nop = mybir.InstNoOp(
                            name=f"I-wsplit-{_wsplit_counter[0]}", ins=[], outs=[])
                        nop.engine = inst.engine
                        nop.sync_info = mybir.SyncInfo(
                            on_wait=overflow[i: i + max_waits], on_update=[])
                        out.append(nop)
                    inst.sync_info = mybir.SyncInfo(
                        on_wait=keep, on_update=list(si.on_update))
                    changed = True
                    n_split += 1
                out.append(inst)
            if changed:
                blk.instructions = out
    return n_split


def _finalize_kernel(nc):
    import bass_rust as _bass_rust
    from concourse.library_config import all_libraries, standard
    from concourse.library_overlay import lower_extended_insts

    inst_type_to_lib_mask = {}
    for lib in all_libraries:
        for inst_type in lib.instructions:
            inst_type_to_lib_mask[inst_type] = inst_type_to_lib_mask.get(
                inst_type, 0) | (1 << lib.index)
    _bass_rust.insert_library_loads(
        nc, inst_type_to_lib_mask, len(all_libraries), standard.index)
    lower_extended_insts(nc)
    _split_excess_waits(nc)


def _bc(ap, *dims):
    """Append free dims [stride, size] to an AP (for broadcast/stride views)."""
    return bass.AP(ap.tensor, ap.offset, list(ap.ap) + [list(d) for d in dims])


# ------------------------------------------------------------------ host prep

class _Graph:
    """Host-side index preprocessing: sort by dst, shard dst windows across
    cores, pad per-window tile counts to a global schedule so all cores run
    one identical SPMD program."""

    def __init__(self, edge_index, n_nodes, n_cores):
        self.N = n_nodes
        self.C = n_cores
        src = np.asarray(edge_index[0], dtype=np.int64)
        dst = np.asarray(edge_index[1], dtype=np.int64)
        perm = np.argsort(dst, kind="stable")
        self.src_s = src[perm].astype(np.int32)
        self.dst_s = dst[perm].astype(np.int32)

        n_win_total = (n_nodes + P - 1) // P
        self.wpc = (n_win_total + n_cores - 1) // n_cores
        self.n_win = self.wpc * n_cores
        self.shard_nodes = self.wpc * P

        bounds = np.searchsorted(self.dst_s, np.arange(0, self.n_win + 1) * P)
        wcnt = bounds[1:] - bounds[:-1]          # edges per window (global)
        # Balance the SPMD schedule: windows sorted by edge count, ranks of 8
        # spread across cores, so PC[i] = max over near-equal counts.
        order = np.argsort(-wcnt, kind="stable")
        self.wmap = np.empty((n_cores, self.wpc), dtype=np.int64)
        for r in range(self.wpc):
            for k in range(n_cores):
                self.wmap[k, r] = order[r * n_cores + k]
        counts = wcnt[self.wmap]                  # [n_cores, wpc]
        self.PC = np.maximum(np.ceil(counts / P).astype(np.int64).max(axis=0), 1)
        self.T = int(self.PC.sum())

        self.slot_src = np.zeros((n_cores, self.T * P), dtype=np.int32)
        self.slot_dst = np.zeros((n_cores, self.T * P), dtype=np.int32)
        self.slot_rel = np.full((n_cores, self.T * P), int(PAD_REL), dtype=np.int32)
        for k in range(n_cores):
            t0 = 0
            for i in range(self.wpc):
                w = int(self.wmap[k, i])
                cnt = int(counts[k, i])
                if cnt > 0:
                    e0 = bounds[w]
                    sl = t0 * P
                    self.slot_src[k, sl:sl + cnt] = self.src_s[e0:e0 + cnt]
                    self.slot_dst[k, sl:sl + cnt] = self.dst_s[e0:e0 + cnt]
                    self.slot_rel[k, sl:sl + cnt] = self.dst_s[e0:e0 + cnt] - w * P
                t0 += int(self.PC[i])
        # rel as [P, T] f16 (col t = rel of edges t*P .. t*P+127)
        self.rel_pt = np.ascontiguousarray(
            self.slot_rel.reshape(n_cores, self.T, P).transpose(0, 2, 1)
        ).astype(np.float16)

    def build_stream(self, table, core, n_src_cols, n_dst_cols):
        """Pack per-edge gathered rows into the DMA-friendly stream layout
        [128][T, C] where C = n_src_cols + n_dst_cols.  The table is
        [n_win*P, Ctab] f16; cols [0:n_src_cols] are gathered by edge
        source, cols [n_src_cols:n_src_cols+n_dst_cols] by edge dest."""
        T, C = self.T, n_src_cols + n_dst_cols
        out = np.empty((P, T, C), dtype=np.float16)
        gs = table[self.slot_src[core], :n_src_cols]
        out[:, :, :n_src_cols] = gs.reshape(T, P, n_src_cols).transpose(1, 0, 2)
        gd = table[self.slot_dst[core], n_src_cols:n_src_cols + n_dst_cols]
        out[:, :, n_src_cols:n_src_cols + n_dst_cols] = (
            gd.reshape(T, P, n_dst_cols).transpose(1, 0, 2))
        return np.ascontiguousarray(out).reshape(P, T * C)

    def stream_S8(self, core):
        if not hasattr(self, "_s8"):
            self._s8 = {}
        if core not in self._s8:
            import ml_dtypes
            rel = self.rel_pt[core].astype(np.int32)        # [P, T]
            onehot = (rel[:, :, None] ==
                      np.arange(P, dtype=np.int32)[None, None, :])
            self._s8[core] = np.ascontiguousarray(
                onehot.astype(ml_dtypes.float8_e4m3)).reshape(P, self.T * P)
        return self._s8[core]

    def unshuffle(self, core_outs):
        """Reassemble per-core window-shuffled outputs into global node order."""
        C = core_outs[0].shape[1]
        out = np.empty((self.n_win * P, C), dtype=core_outs[0].dtype)
        ov = out.reshape(self.n_win, P, C)
        for k in range(self.C):
            ov[self.wmap[k]] = core_outs[k].reshape(self.wpc, P, C)
        return out


# ------------------------------------------------------------------ builders

def _build_node_kernel(wpc, c_in, c_out, bench_loop=1):
    """H = xT.T @ wext per node window; out [wpc*P, c_out] f16."""
    nc = bass.Bass()
    xT = nc.dram_tensor("xT", [c_in, wpc * P], F16, kind="ExternalInput")
    wext = nc.dram_tensor("wext", [c_in, c_out], F16, kind="ExternalInput")
    out = nc.dram_tensor("out", [wpc * P, c_out], F16, kind="ExternalOutput")

    NB = 3
    with tile.TileContext(nc) as tc:
        with (
            tc.tile_pool(name="const", bufs=1) as constp,
            tc.tile_pool(name="x", bufs=2) as xp,
            tc.tile_pool(name="o", bufs=3) as op_,
            tc.tile_pool(name="ps", bufs=2, space="PSUM") as psp,
        ):
            wext_sb = constp.tile([c_in, c_out], F16)
            nc.sync.dma_start(out=wext_sb[:], in_=wext[:])
            def node_phase(_iv=None):
                # whole node shard SBUF-resident; loaded inside the phase so
                # the bench loop counts the full x traffic
                xsh = xp.tile([c_in, wpc * P], F16, tag="xsh")
                nc.sync.dma_start(out=xsh[:], in_=xT[:])
                for c0 in range(0, wpc, NB):
                    nb = min(NB, wpc - c0)
                    ps = psp.tile([P, NB * c_out], F32, tag="ps")
                    for c in range(nb):
                        nc.tensor.matmul(
                            ps[:, c * c_out:(c + 1) * c_out],
                            xsh[:, (c0 + c) * P:(c0 + c + 1) * P], wext_sb[:],
                            start=True, stop=True)
                    ot = op_.tile([P, NB * c_out], F16, tag="ot")
                    nc.scalar.activation(ot[:, :nb * c_out], ps[:, :nb * c_out],
                                         AF.Copy)
                    dst = out[c0 * P:(c0 + nb) * P, :].rearrange(
                        "(c p) f -> p c f", p=P)
                    nc.sync.dma_start(
                        out=dst,
                        in_=ot[:, :nb * c_out].rearrange(
                            "p (c f) -> p c f", c=nb))

            if bench_loop > 1:
                with tc.For_i(0, bench_loop, 1) as _iv:
                    node_phase(_iv)
            else:
                node_phase()
    _finalize_kernel(nc)
    return nc


def _build_edge_kernel(T, PC, wpc, heads, hid, elu, add_bias, fuse_cols,
                       bench_loop=1):
    """Edge phase. Stream cols: [h (HC) | als (heads) | ald (heads)]; the
    one-hot S matrices arrive as a separate fp8 stream (exact for 0/1) and
    feed the PE directly as the segment-sum stationary (fp8 x f16 matmul).
    If fuse_cols > 0: epilogue computes o2 @ w2ext -> out [wpc*P, fuse_cols]
    f16 (layer-2 node phase fused in).  Else out is [wpc*P, HC] f32."""
    HC = heads * hid
    C = HC + 2 * heads
    nc = bass.Bass()
    xs = nc.dram_tensor("xs", [P, T * C], F16, kind="ExternalInput")
    s8 = nc.dram_tensor("s8", [P, T * P], F8, kind="ExternalInput")
    if fuse_cols:
        ident_c = nc.dram_tensor("ident", [P, P], F16, kind="ExternalInput")
        w2ext = nc.dram_tensor("w2ext", [P, fuse_cols], F16, kind="ExternalInput")
        out = nc.dram_tensor("out", [wpc * P, fuse_cols], F16,
                             kind="ExternalOutput")
    else:
        out = nc.dram_tensor("out", [wpc * P, HC], F32, kind="ExternalOutput")
    if add_bias:
        brep = nc.dram_tensor("brep", [P, HC], F32, kind="ExternalInput")

    GRP = CFG["grp"]
    n_groups = (T + GRP - 1) // GRP
    tile_win = []
    for i in range(wpc):
        tile_win += [i] * int(PC[i])
    first_of_win, last_of_win = {}, {}
    for t, w in enumerate(tile_win):
        first_of_win.setdefault(w, t)
        last_of_win[w] = t

    with tile.TileContext(nc) as tc:
        with (
            tc.tile_pool(name="const", bufs=1) as constp,
            tc.tile_pool(name="stream", bufs=3) as streamp,
            tc.tile_pool(name="smat", bufs=2) as sp_,
            tc.tile_pool(name="msg", bufs=2) as msgp,
            tc.tile_pool(name="zexp", bufs=2) as zp,
            tc.tile_pool(name="epi", bufs=2) as epip,
            tc.tile_pool(name="psW", bufs=4, space="PSUM") as psW,
            tc.tile_pool(name="psE", bufs=2, space="PSUM") as psE,
        ):
            if fuse_cols:
                ident_sb = constp.tile([P, P], F16)
                nc.sync.dma_start(out=ident_sb[:], in_=ident_c[:])
                w2_sb = constp.tile([P, fuse_cols], F16)
                nc.sync.dma_start(out=w2_sb[:], in_=w2ext[:])
            if add_bias:
                brep_sb = constp.tile([P, HC], F32)
                nc.sync.dma_start(out=brep_sb[:], in_=brep[:])
            ebias_sb = constp.tile([P, 1], F32)
            nc.vector.memset(ebias_sb[:], EXP_BIAS)

            def epilogue(w, psw):
                epi_eng = nc.gpsimd if CFG["epi_pool"] else nc.vector
                # every real node has a self-loop -> denom > 0; pad rows give
                # inf/NaN that are never gathered downstream
                rec = epip.tile([P, heads], F32, tag="rec")
                nc.vector.reciprocal(rec[:], psw[:, HC:HC + heads])
                o1 = epip.tile([P, HC], F32, tag="o1")
                r_ap = rec[:]
                nc.vector.tensor_tensor(
                    out=o1[:], in0=psw[:, 0:HC],
                    in1=bass.AP(r_ap.tensor, r_ap.offset,
                                [r_ap.ap[0], [1, heads], [0, hid]]),
                    op=OP.mult)
                if add_bias:
                    nc.vector.tensor_tensor(out=o1[:], in0=o1[:],
                                            in1=brep_sb[:], op=OP.add)
                if elu:
                    mn = epip.tile([P, HC], F32, tag="mn")
                    epi_eng.tensor_scalar(mn[:], o1[:], 0.0, None, OP.min)
                    ex = epip.tile([P, HC], F32, tag="ex")
                    nc.scalar.activation(ex[:], mn[:], AF.Exp)
                    mx = epip.tile([P, HC], F32, tag="mx")
                    epi_eng.tensor_scalar(mx[:], o1[:], 0.0, -1.0,
                                          OP.max, OP.add)
                else:
                    ex = mx = None
                if fuse_cols:
                    o2 = epip.tile([P, HC], F16, tag="o2")
                    if elu:
                        nc.vector.tensor_tensor(out=o2[:], in0=mx[:], in1=ex[:],
                                                op=OP.add)
                    else:
                        nc.vector.tensor_copy(o2[:], o1[:])
                    psT = psE.tile([P, P], F16, tag="psT")
                    nc.tensor.transpose(psT[:], o2[:], ident_sb[:])
                    o2T = epip.tile([P, P], F16, tag="o2T")
                    psH = psE.tile([P, fuse_cols], F32, tag="psH")
                    h2 = epip.tile([P, fuse_cols], F16, tag="h2")
                    if CFG["copy_act"]:
                        nc.scalar.activation(o2T[:], psT[:], AF.Copy)
                    else:
                        nc.vector.tensor_copy(o2T[:], psT[:])
                    nc.tensor.matmul(psH[:], o2T[:], w2_sb[:],
                                     start=True, stop=True)
                    if CFG["copy_act"]:
                        nc.scalar.activation(h2[:], psH[:], AF.Copy)
                    else:
                        nc.vector.tensor_copy(h2[:], psH[:])
                    nc.sync.dma_start(out=out[w * P:(w + 1) * P, :], in_=h2[:])
                else:
                    if elu:
                        res = epip.tile([P, HC], F32, tag="res")
                        nc.vector.tensor_tensor(out=res[:], in0=mx[:],
                                                in1=ex[:], op=OP.add)
                    else:
                        res = o1
                    nc.sync.dma_start(out=out[w * P:(w + 1) * P, :], in_=res[:])

            def edge_phase(_iv=None):
                psw_cur = None
                scount = [0]
                for g in range(n_groups):
                    tlo, thi = g * GRP, min(T, g * GRP + GRP)
                    ng = thi - tlo
                    xs_g = streamp.tile([P, GRP, C], F16, tag="xs")
                    nc.sync.dma_start(out=xs_g[:, :ng, :].rearrange(
                        "p g c -> p (g c)"),
                        in_=xs[:, tlo * C:thi * C])
                    S_g = sp_.tile([P, GRP, P], F8, tag="S")
                    nc.sync.dma_start(out=S_g[:, :ng, :].rearrange(
                        "p g c -> p (g c)"),
                        in_=s8[:, tlo * P:thi * P])
                    # z = als + ald  [P, ng, heads] f32
                    zf = zp.tile([P, GRP, heads], F32, tag="zf")
                    nc.vector.tensor_tensor(
                        out=zf[:, :ng, :], in0=xs_g[:, :ng, HC:HC + heads],
                        in1=xs_g[:, :ng, HC + heads:HC + 2 * heads], op=OP.add)
                    nc.scalar.activation(zf[:, :ng, :], zf[:, :ng, :],
                                         AF.Prelu, alpha=NEG_SLOPE)
                    # exp into the denominator columns of msg
                    msg_g = msgp.tile([P, GRP, HC + heads], F16, tag="msg")
                    nc.scalar.activation(msg_g[:, :ng, HC:HC + heads],
                                         zf[:, :ng, :], AF.Exp,
                                         bias=ebias_sb[:])
                    # messages = h * exp
                    ean, ead = CFG["exp_act_num"], CFG["exp_act_den"]
                    if (g % ead) < ean:
                        # ACT expands exp to all HC cols; DVE multiply is
                        # all-packed f16 (2x mode)
                        expf = zp.tile([P, GRP, HC], F16, tag="expf")
                        zin = zf[:, :ng, :]
                        zin_b = bass.AP(zin.tensor, zin.offset,
                                        [zin.ap[0], [heads, ng], [1, heads],
                                         [0, hid]])
                        nc.scalar.activation(expf[:, :ng, :], zin_b, AF.Exp,
                                             bias=ebias_sb[:])
                        nc.vector.tensor_tensor(
                            out=msg_g[:, :ng, 0:HC], in0=xs_g[:, :ng, 0:HC],
                            in1=expf[:, :ng, :], op=OP.mult)
                    else:
                        # DVE multiplies against broadcast exp cols (1x mode)
                        e_ap = msg_g[:, :ng, HC:HC + heads]
                        e_b = bass.AP(e_ap.tensor, e_ap.offset,
                                      [e_ap.ap[0], [HC + heads, ng],
                                       [1, heads], [0, hid]])
                        o_ap = msg_g[:, :ng, 0:HC]
                        o_b = bass.AP(o_ap.tensor, o_ap.offset,
                                      [o_ap.ap[0], [HC + heads, ng],
                                       [hid, heads], [1, hid]])
                        i_ap = xs_g[:, :ng, 0:HC]
                        i_b = bass.AP(i_ap.tensor, i_ap.offset,
                                      [i_ap.ap[0], [C, ng],
                                       [hid, heads], [1, hid]])
                        nc.vector.tensor_tensor(out=o_b, in0=i_b, in1=e_b,
                                                op=OP.mult)
                    for j in range(ng):
                        t = tlo + j
                        w = tile_win[t]
                        if t == first_of_win[w]:
                            psw_cur = psW.tile([P, HC + heads], F32, tag="psW")
                        nc.tensor.matmul(
                            psw_cur[:], S_g[:, j, :], msg_g[:, j, :],
                            start=(t == first_of_win[w]),
                            stop=(t == last_of_win[w]))
                        if t == last_of_win[w]:
                            epilogue(tile_win[t], psw_cur)

            if bench_loop > 1:
                with tc.For_i(0, bench_loop, 1) as _iv:
                    edge_phase(_iv)
            else:
                edge_phase()
    _finalize_kernel(nc)
    return nc


# ------------------------------------------------------------------ runner

def _fold_att(W, a):
    heads, hid = a.shape
    return np.einsum("ihc,hc->ih", W.reshape(W.shape[0], heads, hid), a)


class _GatRunner:
    def __init__(self, n_cores=N_CORES):
        self.C = n_cores
        self._graph = None
        self._graph_key = None
        self._kernels = {}

    def graph(self, edge_index, n_nodes):
        key = hash(np.asarray(edge_index).tobytes())
        if key != self._graph_key:
            self._graph = _Graph(edge_index, n_nodes, self.C)
            self._graph_key = key
            self._kernels.clear()
        return self._graph

    def node_kernel(self, g, c_in, c_out, bench_loop=1):
        key = ("N", g.T, c_in, c_out, bench_loop)
        if key not in self._kernels:
            self._kernels[key] = _build_node_kernel(g.wpc, c_in, c_out,
                                                    bench_loop)
        return self._kernels[key]

    def edge_kernel(self, name, g, heads, hid, elu, add_bias, fuse_cols,
                    bench_loop=1):
        key = (name, g.T, heads, hid, elu, add_bias, fuse_cols, bench_loop)
        if key not in self._kernels:
            self._kernels[key] = _build_edge_kernel(
                g.T, g.PC, g.wpc, heads, hid, elu, add_bias, fuse_cols,
                bench_loop)
        return self._kernels[key]

    @staticmethod
    def w1ext(W1, a_src1, a_dst1):
        return np.concatenate(
            [W1, _fold_att(W1, a_src1), _fold_att(W1, a_dst1)],
            axis=1).astype(np.float16)

    @staticmethod
    def w2ext(W2, a_src2, a_dst2):
        return np.concatenate(
            [W2, _fold_att(W2, a_src2), _fold_att(W2, a_dst2)],
            axis=1).astype(np.float16)

    def node_maps(self, g, x, wextv):
        xT_pad = np.zeros((x.shape[1], g.n_win * P), dtype=np.float16)
        xT_pad[:, :x.shape[0]] = np.asarray(x, np.float32).T
        return [{
            "xT": np.ascontiguousarray(
                xT_pad[:, k * g.shard_nodes:(k + 1) * g.shard_nodes]),
            "wext": wextv,
        } for k in range(self.C)]

    def edge_maps(self, g, table, heads, hid, fuse_w2=None, brep=None):
        HC = heads * hid
        maps = []
        for k in range(self.C):
            im = {
                "xs": g.build_stream(table, k, HC + heads, heads),
                "s8": g.stream_S8(k),
            }
            if fuse_w2 is not None:
                im["ident"] = np.eye(P, dtype=np.float16)
                im["w2ext"] = fuse_w2
            if brep is not None:
                im["brep"] = brep
            maps.append(im)
        return maps

    def run(self, x, edge_index, W1, a_src1, a_dst1, b1, W2, a_src2, a_dst2,
            b2):
        C = self.C
        N, IN_C = x.shape
        HEADS, HID = a_src1.shape
        HC = HEADS * HID
        OUT_C = W2.shape[1]
        g = self.graph(edge_index, N)
        b1nz = bool(np.any(b1))
        b2nz = bool(np.any(b2))
        assert not b1nz and not b2nz, "nonzero biases not wired up"

        w1e = self.w1ext(W1, a_src1, a_dst1)          # [IN_C, HC+2*HEADS]
        w2e = self.w2ext(W2, a_src2, a_dst2)          # [HC, OUT_C+2]

        ncN = self.node_kernel(g, IN_C, w1e.shape[1])
        resN = run_bass_kernel_spmd(ncN, self.node_maps(g, x, w1e),
                                    core_ids=list(range(C)))
        table1 = np.concatenate([r["out"] for r in resN.results], axis=0)

        ncE1 = self.edge_kernel("E1", g, HEADS, HID, True, False,
                                w2e.shape[1])
        mapsE1 = self.edge_maps(g, table1, HEADS, HID, fuse_w2=w2e)
        resE1 = run_bass_kernel_spmd(ncE1, mapsE1, core_ids=list(range(C)))
        table2 = g.unshuffle([r["out"] for r in resE1.results])

        ncE2 = self.edge_kernel("E2", g, 1, OUT_C, False, False, 0)
        mapsE2 = self.edge_maps(g, table2, 1, OUT_C)
        resE2 = run_bass_kernel_spmd(ncE2, mapsE2, core_ids=list(range(C)))
        return g.unshuffle([r["out"] for r in resE2.results])[:N]


_RUNNER = _GatRunner()


def kernel(x, edge_index, W1, a_src1, a_dst1, b1, W2, a_src2, a_dst2, b2):
    """Full-input / full-output entry point. Returns [N, OUT_C] float32."""
    args = [np.asarray(v) for v in
            (x, edge_index, W1, a_src1, a_dst1, b1, W2, a_src2, a_dst2, b2)]
    return _RUNNER.run(*args).astype(np.float32)


# revision 36
# speedup vs baseline: 1.1084x; 1.1084x over previous
"""Trainium (trn2) Bass kernel for a 2-layer GAT over N=100k nodes / E=1.7M edges.

Strategy (v2 — gather-streamed edge phase)
------------------------------------------
Edges are sorted by destination on the host (index-only preprocessing); the
destination axis is sharded across the 8 NeuronCores in contiguous 128-node
windows (98 per core).  Three SPMD kernels per forward pass:

* N1 (node phase): H1ext = x @ [W1 | W1.a_src | W1.a_dst]  -> [N, 144] f16
  table, node windows sharded across cores.
* host (permutation/cast only, no FLOPs): gather H1ext rows by edge source
  (h + a_src.h) and by edge destination (a_dst.h), pack them together with
  the relative-destination column into a DMA-friendly per-core stream laid
  out [128 partitions][T tiles, C cols] so every partition reads long
  contiguous runs.
* E1 (edge phase L1): per 128-edge tile: z = als+ald; Prelu; one batched
  Exp expanded to all 128 message columns (so the message multiply is an
  all-SBUF packed-f16 TensorTensor in 2x DVE mode); the one-hot
  S[e,n]=(dst_rel==n) arrives as a host-built fp8 stream (exact for 0/1)
  and feeds the PE directly as the stationary of ONE fp8xf16 matmul per
  tile that accumulates both the segment sum and the softmax denominators
  (exp rides as 8 extra message columns).  The per-window epilogue divides
  by the denominator, applies ELU, and fuses layer 2's node matmul (PE
  transpose + o2 @ [W2 | W2.a_src2 | W2.a_dst2]) so E2 only needs
  66-column gathers.
* E2 (edge phase L2): same structure with 1 head / 64 channels; outputs the
  final [N, 64] f32.

All floating-point work runs on device; the host only sorts/gathers/casts.
The NEFF is specialized to the edge distribution and cached.

Environment workarounds: this walrus build allows only ONE semaphore wait
per instruction (split onto nop carriers post-scheduling), and the GPSIMD
ucode libraries are absent (no dma_gather/indirect-DMA fast paths - hence
the host-gathered streams).
"""
import sys
import os
import time

import numpy as np

import concourse.bass as bass
import concourse.mybir as mybir
import concourse.tile as tile
from concourse.bass_utils import run_bass_kernel_spmd

P = 128
F16 = mybir.dt.float16
F32 = mybir.dt.float32
F8 = mybir.dt.float8e4
AF = mybir.ActivationFunctionType
OP = mybir.AluOpType
NEG_SLOPE = 0.2
EXP_BIAS = -4.0     # exp(z + EXP_BIAS): constant shift cancels in softmax
GRP = 32            # tiles per stream group
PAD_REL = 255.0     # rel value for pad slots -> is_equal never matches
N_CORES = 8

# engine-assignment tuning knobs (read at kernel-build time)
# NOTE: Pool/GPSIMD offload measured ~10x slower on real HW than the
# TimelineSim cost model predicts (software Q7 ucode) - keep everything off
# the Pool engine.
CFG = {
    "pool_s_num": 0,      # pool_s_num of every pool_s_den S-builds on Pool
    "pool_s_den": 8,
    "z_add_pool": False,  # z = als+ald on Pool instead of DVE
    "epi_pool": False,    # ELU min/max on Pool
    "copy_act": True,     # epilogue PSUM->SBUF copies on ACT (Copy act)
    "grp": 32,            # tiles per stream group
    # exp expansion path per group: exp_act_num of every exp_act_den groups
    # use ACT-expanded exp + packed DVE multiply; the rest multiply against
    # a broadcast exp AP directly on DVE (slower per element, no ACT cost)
    "exp_act_num": 8,
    "exp_act_den": 8,
    "bufs": 2,            # double/triple buffering of S/msg/zexp pools
    "psw_bufs": 4,
    "grp_e2": 64,         # E2 is overhead-bound; bigger groups amortize better
}

# ------------------------------------------------------------------ patches

_wsplit_counter = [0]


def _split_excess_waits(nc, max_waits=1):
    """This walrus build rejects >1 sem-wait per instruction ("Too many sync
    wait commands"). Move overflow waits onto same-engine nop carriers."""
    n_split = 0
    for f in nc.m.functions:
        for blk in f.blocks:
            changed = False
            out = []
            for inst in blk.instructions:
                si = inst.sync_info
                if si is not None and len(si.on_wait) > max_waits:
                    waits = list(si.on_wait)
                    keep = waits[len(waits) - max_waits:]
                    overflow = waits[: len(waits) - max_waits]
                    for i in range(0, len(overflow), max_waits):
                        _wsplit_counter[0] += 1
                        nop = mybir.InstNoOp(
                            name=f"I-wsplit-{_wsplit_counter[0]}", ins=[], outs=[])
                        nop.engine = inst.engine
                        nop.sync_info = mybir.SyncInfo(
                            on_wait=overflow[i: i + max_waits], on_update=[])
                        out.append(nop)
                    inst.sync_info = mybir.SyncInfo(
                        on_wait=keep, on_update=list(si.on_update))
                    changed = True
                    n_split += 1
                out.append(inst)
            if changed:
                blk.instructions = out
    return n_split


def _finalize_kernel(nc):
    import bass_rust as _bass_rust
    from concourse.library_config import all_libraries, standard
    from concourse.library_overlay import lower_extended_insts

    inst_type_to_lib_mask = {}
    for lib in all_libraries:
        for inst_type in lib.instructions:
            inst_type_to_lib_mask[inst_type] = inst_type_to_lib_mask.get(
                inst_type, 0) | (1 << lib.index)
    _bass_rust.insert_library_loads(
        nc, inst_type_to_lib_mask, len(all_libraries), standard.index)
    lower_extended_insts(nc)
    _split_excess_waits(nc)


def _bc(ap, *dims):
    """Append free dims [stride, size] to an AP (for broadcast/stride views)."""
    return bass.AP(ap.tensor, ap.offset, list(ap.ap) + [list(d) for d in dims])


# ------------------------------------------------------------------ host prep

class _Graph:
    """Host-side index preprocessing: sort by dst, shard dst windows across
    cores, pad per-window tile counts to a global schedule so all cores run
    one identical SPMD program."""

    def __init__(self, edge_index, n_nodes, n_cores):
        self.N = n_nodes
        self.C = n_cores
        src = np.asarray(edge_index[0], dtype=np.int64)
        dst = np.asarray(edge_index[1], dtype=np.int64)
        perm = np.argsort(dst, kind="stable")
        self.src_s = src[perm].astype(np.int32)
        self.dst_s = dst[perm].astype(np.int32)

        n_win_total = (n_nodes + P - 1) // P
        self.wpc = (n_win_total + n_cores - 1) // n_cores
        self.n_win = self.wpc * n_cores
        self.shard_nodes = self.wpc * P

        bounds = np.searchsorted(self.dst_s, np.arange(0, self.n_win + 1) * P)
        wcnt = bounds[1:] - bounds[:-1]          # edges per window (global)
        # Balance the SPMD schedule: windows sorted by edge count, ranks of 8
        # spread across cores, so PC[i] = max over near-equal counts.
        order = np.argsort(-wcnt, kind="stable")
        self.wmap = np.empty((n_cores, self.wpc), dtype=np.int64)
        for r in range(self.wpc):
            for k in range(n_cores):
                self.wmap[k, r] = order[r * n_cores + k]
        counts = wcnt[self.wmap]                  # [n_cores, wpc]
        self.PC = np.maximum(np.ceil(counts / P).astype(np.int64).max(axis=0), 1)
        self.T = int(self.PC.sum())

        self.slot_src = np.zeros((n_cores, self.T * P), dtype=np.int32)
        self.slot_dst = np.zeros((n_cores, self.T * P), dtype=np.int32)
        self.slot_rel = np.full((n_cores, self.T * P), int(PAD_REL), dtype=np.int32)
        for k in range(n_cores):
            t0 = 0
            for i in range(self.wpc):
                w = int(self.wmap[k, i])
                cnt = int(counts[k, i])
                if cnt > 0:
                    e0 = bounds[w]
                    sl = t0 * P
                    self.slot_src[k, sl:sl + cnt] = self.src_s[e0:e0 + cnt]
                    self.slot_dst[k, sl:sl + cnt] = self.dst_s[e0:e0 + cnt]
                    self.slot_rel[k, sl:sl + cnt] = self.dst_s[e0:e0 + cnt] - w * P
                t0 += int(self.PC[i])
        # rel as [P, T] f16 (col t = rel of edges t*P .. t*P+127)
        self.rel_pt = np.ascontiguousarray(
            self.slot_rel.reshape(n_cores, self.T, P).transpose(0, 2, 1)
        ).astype(np.float16)

    def build_stream(self, table, core, n_src_cols, n_dst_cols):
        """Pack per-edge gathered rows into the DMA-friendly stream layout
        [128][T, C] where C = n_src_cols + n_dst_cols.  The table is
        [n_win*P, Ctab] f16; cols [0:n_src_cols] are gathered by edge
        source, cols [n_src_cols:n_src_cols+n_dst_cols] by edge dest."""
        T, C = self.T, n_src_cols + n_dst_cols
        out = np.empty((P, T, C), dtype=np.float16)
        gs = table[self.slot_src[core], :n_src_cols]
        out[:, :, :n_src_cols] = gs.reshape(T, P, n_src_cols).transpose(1, 0, 2)
        gd = table[self.slot_dst[core], n_src_cols:n_src_cols + n_dst_cols]
        out[:, :, n_src_cols:n_src_cols + n_dst_cols] = (
            gd.reshape(T, P, n_dst_cols).transpose(1, 0, 2))
        return np.ascontiguousarray(out).reshape(P, T * C)

    def stream_S8(self, core):
        if not hasattr(self, "_s8"):
            self._s8 = {}
        if core not in self._s8:
            import ml_dtypes
            rel = self.rel_pt[core].astype(np.int32)        # [P, T]
            onehot = (rel[:, :, None] ==
                      np.arange(P, dtype=np.int32)[None, None, :])
            self._s8[core] = np.ascontiguousarray(
                onehot.astype(ml_dtypes.float8_e4m3)).reshape(P, self.T * P)
        return self._s8[core]

    def unshuffle(self, core_outs):
        """Reassemble per-core window-shuffled outputs into global node order."""
        C = core_outs[0].shape[1]
        out = np.empty((self.n_win * P, C), dtype=core_outs[0].dtype)
        ov = out.reshape(self.n_win, P, C)
        for k in range(self.C):
            ov[self.wmap[k]] = core_outs[k].reshape(self.wpc, P, C)
        return out


# ------------------------------------------------------------------ builders

def _build_node_kernel(wpc, c_in, c_out, bench_loop=1):
    """H = xT.T @ wext per node window; out [wpc*P, c_out] f16."""
    nc = bass.Bass()
    xT = nc.dram_tensor("xT", [c_in, wpc * P], F16, kind="ExternalInput")
    wext = nc.dram_tensor("wext", [c_in, c_out], F16, kind="ExternalInput")
    out = nc.dram_tensor("out", [wpc * P, c_out], F16, kind="ExternalOutput")

    NB = 3
    with tile.TileContext(nc) as tc:
        with (
            tc.tile_pool(name="const", bufs=1) as constp,
            tc.tile_pool(name="x", bufs=2) as xp,
            tc.tile_pool(name="o", bufs=3) as op_,
            tc.tile_pool(name="ps", bufs=2, space="PSUM") as psp,
        ):
            wext_sb = constp.tile([c_in, c_out], F16)
            nc.sync.dma_start(out=wext_sb[:], in_=wext[:])
            def node_phase(_iv=None):
                # whole node shard SBUF-resident; loaded inside the phase so
                # the bench loop counts the full x traffic
                xsh = xp.tile([c_in, wpc * P], F16, tag="xsh")
                nc.sync.dma_start(out=xsh[:], in_=xT[:])
                for c0 in range(0, wpc, NB):
                    nb = min(NB, wpc - c0)
                    ps = psp.tile([P, NB * c_out], F32, tag="ps")
                    for c in range(nb):
                        nc.tensor.matmul(
                            ps[:, c * c_out:(c + 1) * c_out],
                            xsh[:, (c0 + c) * P:(c0 + c + 1) * P], wext_sb[:],
                            start=True, stop=True)
                    ot = op_.tile([P, NB * c_out], F16, tag="ot")
                    nc.scalar.activation(ot[:, :nb * c_out], ps[:, :nb * c_out],
                                         AF.Copy)
                    dst = out[c0 * P:(c0 + nb) * P, :].rearrange(
                        "(c p) f -> p c f", p=P)
                    nc.sync.dma_start(
                        out=dst,
                        in_=ot[:, :nb * c_out].rearrange(
                            "p (c f) -> p c f", c=nb))

            if bench_loop > 1:
                with tc.For_i(0, bench_loop, 1) as _iv:
                    node_phase(_iv)
            else:
                node_phase()
    _finalize_kernel(nc)
    return nc


def _build_edge_kernel(T, PC, wpc, heads, hid, elu, add_bias, fuse_cols,
                       bench_loop=1):
    """Edge phase. Stream cols: [h (HC) | als (heads) | ald (heads)]; the
    one-hot S matrices arrive as a separate fp8 stream (exact for 0/1) and
    feed the PE directly as the segment-sum stationary (fp8 x f16 matmul).
    If fuse_cols > 0: epilogue computes o2 @ w2ext -> out [wpc*P, fuse_cols]
    f16 (layer-2 node phase fused in).  Else out is [wpc*P, HC] f32."""
    HC = heads * hid
    C = HC + 2 * heads
    nc = bass.Bass()
    xs = nc.dram_tensor("xs", [P, T * C], F16, kind="ExternalInput")
    s8 = nc.dram_tensor("s8", [P, T * P], F8, kind="ExternalInput")
    if fuse_cols:
        ident_c = nc.dram_tensor("ident", [P, P], F16, kind="ExternalInput")
        w2ext = nc.dram_tensor("w2ext", [P, fuse_cols], F16, kind="ExternalInput")
        out = nc.dram_tensor("out", [wpc * P, fuse_cols], F16,
                             kind="ExternalOutput")
    else:
        out = nc.dram_tensor("out", [wpc * P, HC], F32, kind="ExternalOutput")
    if add_bias:
        brep = nc.dram_tensor("brep", [P, HC], F32, kind="ExternalInput")

    GRP = CFG["grp"]
    n_groups = (T + GRP - 1) // GRP
    tile_win = []
    for i in range(wpc):
        tile_win += [i] * int(PC[i])
    first_of_win, last_of_win = {}, {}
    for t, w in enumerate(tile_win):
        first_of_win.setdefault(w, t)
        last_of_win[w] = t

    with tile.TileContext(nc) as tc:
        with (
            tc.tile_pool(name="const", bufs=1) as constp,
            tc.tile_pool(name="stream", bufs=3) as streamp,
            tc.tile_pool(name="smat", bufs=CFG["bufs"]) as sp_,
            tc.tile_pool(name="msg", bufs=CFG["bufs"]) as msgp,
            tc.tile_pool(name="zexp", bufs=CFG["bufs"]) as zp,
            tc.tile_pool(name="epi", bufs=2) as epip,
            tc.tile_pool(name="psW", bufs=CFG["psw_bufs"], space="PSUM") as psW,
            tc.tile_pool(name="psE", bufs=2, space="PSUM") as psE,
        ):
            if fuse_cols:
                ident_sb = constp.tile([P, P], F16)
                nc.sync.dma_start(out=ident_sb[:], in_=ident_c[:])
                w2_sb = constp.tile([P, fuse_cols], F16)
                nc.sync.dma_start(out=w2_sb[:], in_=w2ext[:])
            if add_bias:
                brep_sb = constp.tile([P, HC], F32)
                nc.sync.dma_start(out=brep_sb[:], in_=brep[:])
            ebias_sb = constp.tile([P, 1], F32)
            nc.vector.memset(ebias_sb[:], EXP_BIAS)

            def epilogue(w, psw):
                epi_eng = nc.gpsimd if CFG["epi_pool"] else nc.vector
                # every real node has a self-loop -> denom > 0; pad rows give
                # inf/NaN that are never gathered downstream
                rec = epip.tile([P, heads], F32, tag="rec")
                nc.vector.reciprocal(rec[:], psw[:, HC:HC + heads])
                o1 = epip.tile([P, HC], F32, tag="o1")
                r_ap = rec[:]
                nc.vector.tensor_tensor(
                    out=o1[:], in0=psw[:, 0:HC],
                    in1=bass.AP(r_ap.tensor, r_ap.offset,
                                [r_ap.ap[0], [1, heads], [0, hid]]),
                    op=OP.mult)
                if add_bias:
                    nc.vector.tensor_tensor(out=o1[:], in0=o1[:],
                                            in1=brep_sb[:], op=OP.add)
                if elu:
                    mn = epip.tile([P, HC], F32, tag="mn")
                    epi_eng.tensor_scalar(mn[:], o1[:], 0.0, None, OP.min)
                    ex = epip.tile([P, HC], F32, tag="ex")
                    nc.scalar.activation(ex[:], mn[:], AF.Exp)
                    mx = epip.tile([P, HC], F32, tag="mx")
                    epi_eng.tensor_scalar(mx[:], o1[:], 0.0, -1.0,
                                          OP.max, OP.add)
                else:
                    ex = mx = None
                if fuse_cols:
                    o2 = epip.tile([P, HC], F16, tag="o2")
                    if elu:
                        nc.vector.tensor_tensor(out=o2[:], in0=mx[:], in1=ex[:],
                                                op=OP.add)
                    else:
                        nc.vector.tensor_copy(o2[:], o1[:])
                    psT = psE.tile([P, P], F16, tag="psT")
                    nc.tensor.transpose(psT[:], o2[:], ident_sb[:])
                    o2T = epip.tile([P, P], F16, tag="o2T")
                    psH = psE.tile([P, fuse_cols], F32, tag="psH")
                    h2 = epip.tile([P, fuse_cols], F16, tag="h2")
                    if CFG["copy_act"]:
                        nc.scalar.activation(o2T[:], psT[:], AF.Copy)
                    else:
                        nc.vector.tensor_copy(o2T[:], psT[:])
                    nc.tensor.matmul(psH[:], o2T[:], w2_sb[:],
                                     start=True, stop=True)
                    if CFG["copy_act"]:
                        nc.scalar.activation(h2[:], psH[:], AF.Copy)
                    else:
                        nc.vector.tensor_copy(h2[:], psH[:])
                    nc.sync.dma_start(out=out[w * P:(w + 1) * P, :], in_=h2[:])
                else:
                    if elu:
                        res = epip.tile([P, HC], F32, tag="res")
                        nc.vector.tensor_tensor(out=res[:], in0=mx[:],
                                                in1=ex[:], op=OP.add)
                    else:
                        res = o1
                    nc.sync.dma_start(out=out[w * P:(w + 1) * P, :], in_=res[:])

            def edge_phase(_iv=None):
                psw_cur = None
                scount = [0]
                for g in range(n_groups):
                    tlo, thi = g * GRP, min(T, g * GRP + GRP)
                    ng = thi - tlo
                    xs_g = streamp.tile([P, GRP, C], F16, tag="xs")
                    nc.sync.dma_start(out=xs_g[:, :ng, :].rearrange(
                        "p g c -> p (g c)"),
                        in_=xs[:, tlo * C:thi * C])
                    S_g = sp_.tile([P, GRP, P], F8, tag="S")
                    nc.sync.dma_start(out=S_g[:, :ng, :].rearrange(
                        "p g c -> p (g c)"),
                        in_=s8[:, tlo * P:thi * P])
                    # z = als + ald  [P, ng, heads] f32
                    zf = zp.tile([P, GRP, heads], F32, tag="zf")
                    nc.vector.tensor_tensor(
                        out=zf[:, :ng, :], in0=xs_g[:, :ng, HC:HC + heads],
                        in1=xs_g[:, :ng, HC + heads:HC + 2 * heads], op=OP.add)
                    nc.scalar.activation(zf[:, :ng, :], zf[:, :ng, :],
                                         AF.Prelu, alpha=NEG_SLOPE)
                    # exp into the denominator columns of msg
                    msg_g = msgp.tile([P, GRP, HC + heads], F16, tag="msg")
                    nc.scalar.activation(msg_g[:, :ng, HC:HC + heads],
                                         zf[:, :ng, :], AF.Exp,
                                         bias=ebias_sb[:])
                    # messages = h * exp
                    ean, ead = CFG["exp_act_num"], CFG["exp_act_den"]
                    if (g % ead) < ean:
                        # ACT expands exp to all HC cols; DVE multiply is
                        # all-packed f16 (2x mode)
                        expf = zp.tile([P, GRP, HC], F16, tag="expf")
                        zin = zf[:, :ng, :]
                        zin_b = bass.AP(zin.tensor, zin.offset,
                                        [zin.ap[0], [heads, ng], [1, heads],
                                         [0, hid]])
                        nc.scalar.activation(expf[:, :ng, :], zin_b, AF.Exp,
                                             bias=ebias_sb[:])
                        nc.vector.tensor_tensor(
                            out=msg_g[:, :ng, 0:HC], in0=xs_g[:, :ng, 0:HC],
                            in1=expf[:, :ng, :], op=OP.mult)
                    else:
                        # DVE multiplies against broadcast exp cols (1x mode)
                        e_ap = msg_g[:, :ng, HC:HC + heads]
                        e_b = bass.AP(e_ap.tensor, e_ap.offset,
                                      [e_ap.ap[0], [HC + heads, ng],
                                       [1, heads], [0, hid]])
                        o_ap = msg_g[:, :ng, 0:HC]
                        o_b = bass.AP(o_ap.tensor, o_ap.offset,
                                      [o_ap.ap[0], [HC + heads, ng],
                                       [hid, heads], [1, hid]])
                        i_ap = xs_g[:, :ng, 0:HC]
                        i_b = bass.AP(i_ap.tensor, i_ap.offset,
                                      [i_ap.ap[0], [C, ng],
                                       [hid, heads], [1, hid]])
                        nc.vector.tensor_tensor(out=o_b, in0=i_b, in1=e_b,
                                                op=OP.mult)
                    for j in range(ng):
                        t = tlo + j
                        w = tile_win[t]
                        if t == first_of_win[w]:
                            psw_cur = psW.tile([P, HC + heads], F32, tag="psW")
                        nc.tensor.matmul(
                            psw_cur[:], S_g[:, j, :], msg_g[:, j, :],
                            start=(t == first_of_win[w]),
                            stop=(t == last_of_win[w]))
                        if t == last_of_win[w]:
                            epilogue(tile_win[t], psw_cur)

            if bench_loop > 1:
                with tc.For_i(0, bench_loop, 1) as _iv:
                    edge_phase(_iv)
            else:
                edge_phase()
    _finalize_kernel(nc)
    return nc


# ------------------------------------------------------------------ runner

def _fold_att(W, a):
    heads, hid = a.shape
    return np.einsum("ihc,hc->ih", W.reshape(W.shape[0], heads, hid), a)


class _GatRunner:
    def __init__(self, n_cores=N_CORES):
        self.C = n_cores
        self._graph = None
        self._graph_key = None
        self._kernels = {}

    def graph(self, edge_index, n_nodes):
        key = hash(np.asarray(edge_index).tobytes())
        if key != self._graph_key:
            self._graph = _Graph(edge_index, n_nodes, self.C)
            self._graph_key = key
            self._kernels.clear()
        return self._graph

    def node_kernel(self, g, c_in, c_out, bench_loop=1):
        key = ("N", g.T, c_in, c_out, bench_loop)
        if key not in self._kernels:
            self._kernels[key] = _build_node_kernel(g.wpc, c_in, c_out,
                                                    bench_loop)
        return self._kernels[key]

    def edge_kernel(self, name, g, heads, hid, elu, add_bias, fuse_cols,
                    bench_loop=1):
        key = (name, g.T, heads, hid, elu, add_bias, fuse_cols, bench_loop)
        if key not in self._kernels:
            old_grp = CFG["grp"]
            if name == "E2" and CFG.get("grp_e2"):
                CFG["grp"] = CFG["grp_e2"]
            try:
                self._kernels[key] = _build_edge_kernel(
                    g.T, g.PC, g.wpc, heads, hid, elu, add_bias, fuse_cols,
                    bench_loop)
            finally:
                CFG["grp"] = old_grp
        return self._kernels[key]

    @staticmethod
    def w1ext(W1, a_src1, a_dst1):
        return np.concatenate(
            [W1, _fold_att(W1, a_src1), _fold_att(W1, a_dst1)],
            axis=1).astype(np.float16)

    @staticmethod
    def w2ext(W2, a_src2, a_dst2):
        return np.concatenate(
            [W2, _fold_att(W2, a_src2), _fold_att(W2, a_dst2)],
            axis=1).astype(np.float16)

    def node_maps(self, g, x, wextv):
        xT_pad = np.zeros((x.shape[1], g.n_win * P), dtype=np.float16)
        xT_pad[:, :x.shape[0]] = np.asarray(x, np.float32).T
        return [{
            "xT": np.ascontiguousarray(
                xT_pad[:, k * g.shard_nodes:(k + 1) * g.shard_nodes]),
            "wext": wextv,
        } for k in range(self.C)]

    def edge_maps(self, g, table, heads, hid, fuse_w2=None, brep=None):
        HC = heads * hid
        maps = []
        for k in range(self.C):
            im = {
                "xs": g.build_stream(table, k, HC + heads, heads),
                "s8": g.stream_S8(k),
            }
            if fuse_w2 is not None:
                im["ident"] = np.eye(P, dtype=np.float16)
                im["w2ext"] = fuse_w2
            if brep is not None:
                im["brep"] = brep
            maps.append(im)
        return maps

    def run(self, x, edge_index, W1, a_src1, a_dst1, b1, W2, a_src2, a_dst2,
            b2):
        C = self.C
        N, IN_C = x.shape
        HEADS, HID = a_src1.shape
        HC = HEADS * HID
        OUT_C = W2.shape[1]
        g = self.graph(edge_index, N)
        b1nz = bool(np.any(b1))
        b2nz = bool(np.any(b2))
        assert not b1nz and not b2nz, "nonzero biases not wired up"

        w1e = self.w1ext(W1, a_src1, a_dst1)          # [IN_C, HC+2*HEADS]
        w2e = self.w2ext(W2, a_src2, a_dst2)          # [HC, OUT_C+2]

        ncN = self.node_kernel(g, IN_C, w1e.shape[1])
        resN = run_bass_kernel_spmd(ncN, self.node_maps(g, x, w1e),
                                    core_ids=list(range(C)))
        table1 = np.concatenate([r["out"] for r in resN.results], axis=0)

        ncE1 = self.edge_kernel("E1", g, HEADS, HID, True, False,
                                w2e.shape[1])
        mapsE1 = self.edge_maps(g, table1, HEADS, HID, fuse_w2=w2e)
        resE1 = run_bass_kernel_spmd(ncE1, mapsE1, core_ids=list(range(C)))
        table2 = g.unshuffle([r["out"] for r in resE1.results])

        ncE2 = self.edge_kernel("E2", g, 1, OUT_C, False, False, 0)
        mapsE2 = self.edge_maps(g, table2, 1, OUT_C)
        resE2 = run_bass_kernel_spmd(ncE2, mapsE2, core_ids=list(range(C)))
        return g.unshuffle([r["out"] for r in resE2.results])[:N]


_RUNNER = _GatRunner()


def kernel(x, edge_index, W1, a_src1, a_dst1, b1, W2, a_src2, a_dst2, b2):
    """Full-input / full-output entry point. Returns [N, OUT_C] float32."""
    args = [np.asarray(v) for v in
            (x, edge_index, W1, a_src1, a_dst1, b1, W2, a_src2, a_dst2, b2)]
    return _RUNNER.run(*args).astype(np.float32)


# revision 37
# speedup vs baseline: 1.1784x; 1.0631x over previous
"""Trainium (trn2) Bass kernel for a 2-layer GAT over N=100k nodes / E=1.7M edges.

Strategy (v2 — gather-streamed edge phase)
------------------------------------------
Edges are sorted by destination on the host (index-only preprocessing); the
destination axis is sharded across the 8 NeuronCores in contiguous 128-node
windows (98 per core).  Three SPMD kernels per forward pass:

* N1 (node phase): H1ext = x @ [W1 | W1.a_src | W1.a_dst]  -> [N, 144] f16
  table, node windows sharded across cores.
* host (permutation/cast only, no FLOPs): gather H1ext rows by edge source
  (h + a_src.h) and by edge destination (a_dst.h), pack them together with
  the relative-destination column into a DMA-friendly per-core stream laid
  out [128 partitions][T tiles, C cols] so every partition reads long
  contiguous runs.
* E1 (edge phase L1): per 128-edge tile: z = als+ald; Prelu; one batched
  Exp expanded to all 128 message columns (so the message multiply is an
  all-SBUF packed-f16 TensorTensor in 2x DVE mode); the one-hot
  S[e,n]=(dst_rel==n) arrives as a host-built fp8 stream (exact for 0/1)
  and feeds the PE directly as the stationary of ONE fp8xf16 matmul per
  tile that accumulates both the segment sum and the softmax denominators
  (exp rides as 8 extra message columns).  The per-window epilogue divides
  by the denominator, applies ELU, and fuses layer 2's node matmul (PE
  transpose + o2 @ [W2 | W2.a_src2 | W2.a_dst2]) so E2 only needs
  66-column gathers.
* E2 (edge phase L2): same structure with 1 head / 64 channels; outputs the
  final [N, 64] f32.

All floating-point work runs on device; the host only sorts/gathers/casts.
The NEFF is specialized to the edge distribution and cached.

Environment workarounds: this walrus build allows only ONE semaphore wait
per instruction (split onto nop carriers post-scheduling), and the GPSIMD
ucode libraries are absent (no dma_gather/indirect-DMA fast paths - hence
the host-gathered streams).
"""
import sys
import os
import time

import numpy as np

import concourse.bass as bass
import concourse.mybir as mybir
import concourse.tile as tile
from concourse.bass_utils import run_bass_kernel_spmd

P = 128
F16 = mybir.dt.float16
F32 = mybir.dt.float32
F8 = mybir.dt.float8e4
AF = mybir.ActivationFunctionType
OP = mybir.AluOpType
NEG_SLOPE = 0.2
EXP_BIAS = -4.0     # exp(z + EXP_BIAS): constant shift cancels in softmax
GRP = 32            # tiles per stream group
PAD_REL = 255.0     # rel value for pad slots -> is_equal never matches
N_CORES = 8

# engine-assignment tuning knobs (read at kernel-build time)
# NOTE: Pool/GPSIMD offload measured ~10x slower on real HW than the
# TimelineSim cost model predicts (software Q7 ucode) - keep everything off
# the Pool engine.
CFG = {
    "pool_s_num": 0,      # pool_s_num of every pool_s_den S-builds on Pool
    "pool_s_den": 8,
    "z_add_pool": False,  # z = als+ald on Pool instead of DVE
    "epi_pool": False,    # ELU min/max on Pool
    "copy_act": True,     # epilogue PSUM->SBUF copies on ACT (Copy act)
    "grp": 48,            # tiles per stream group (E1)
    # exp expansion path per group: exp_act_num of every exp_act_den groups
    # use ACT-expanded exp + packed DVE multiply; the rest multiply against
    # a broadcast exp AP directly on DVE (slower per element, no ACT cost)
    "exp_act_num": 8,
    "exp_act_den": 8,
    "bufs": 2,            # double/triple buffering of S/msg/zexp pools
    "psw_bufs": 4,
    "grp_e2": 64,         # E2 is overhead-bound; bigger groups amortize better
}

# ------------------------------------------------------------------ patches

_wsplit_counter = [0]


def _split_excess_waits(nc, max_waits=1):
    """This walrus build rejects >1 sem-wait per instruction ("Too many sync
    wait commands"). Move overflow waits onto same-engine nop carriers."""
    n_split = 0
    for f in nc.m.functions:
        for blk in f.blocks:
            changed = False
            out = []
            for inst in blk.instructions:
                si = inst.sync_info
                if si is not None and len(si.on_wait) > max_waits:
                    waits = list(si.on_wait)
                    keep = waits[len(waits) - max_waits:]
                    overflow = waits[: len(waits) - max_waits]
                    for i in range(0, len(overflow), max_waits):
                        _wsplit_counter[0] += 1
                        nop = mybir.InstNoOp(
                            name=f"I-wsplit-{_wsplit_counter[0]}", ins=[], outs=[])
                        nop.engine = inst.engine
                        nop.sync_info = mybir.SyncInfo(
                            on_wait=overflow[i: i + max_waits], on_update=[])
                        out.append(nop)
                    inst.sync_info = mybir.SyncInfo(
                        on_wait=keep, on_update=list(si.on_update))
                    changed = True
                    n_split += 1
                out.append(inst)
            if changed:
                blk.instructions = out
    return n_split


def _finalize_kernel(nc):
    import bass_rust as _bass_rust
    from concourse.library_config import all_libraries, standard
    from concourse.library_overlay import lower_extended_insts

    inst_type_to_lib_mask = {}
    for lib in all_libraries:
        for inst_type in lib.instructions:
            inst_type_to_lib_mask[inst_type] = inst_type_to_lib_mask.get(
                inst_type, 0) | (1 << lib.index)
    _bass_rust.insert_library_loads(
        nc, inst_type_to_lib_mask, len(all_libraries), standard.index)
    lower_extended_insts(nc)
    _split_excess_waits(nc)


def _bc(ap, *dims):
    """Append free dims [stride, size] to an AP (for broadcast/stride views)."""
    return bass.AP(ap.tensor, ap.offset, list(ap.ap) + [list(d) for d in dims])


# ------------------------------------------------------------------ host prep

class _Graph:
    """Host-side index preprocessing: sort by dst, shard dst windows across
    cores, pad per-window tile counts to a global schedule so all cores run
    one identical SPMD program."""

    def __init__(self, edge_index, n_nodes, n_cores):
        self.N = n_nodes
        self.C = n_cores
        src = np.asarray(edge_index[0], dtype=np.int64)
        dst = np.asarray(edge_index[1], dtype=np.int64)
        perm = np.argsort(dst, kind="stable")
        self.src_s = src[perm].astype(np.int32)
        self.dst_s = dst[perm].astype(np.int32)

        n_win_total = (n_nodes + P - 1) // P
        self.wpc = (n_win_total + n_cores - 1) // n_cores
        self.n_win = self.wpc * n_cores
        self.shard_nodes = self.wpc * P

        bounds = np.searchsorted(self.dst_s, np.arange(0, self.n_win + 1) * P)
        wcnt = bounds[1:] - bounds[:-1]          # edges per window (global)
        # Balance the SPMD schedule: windows sorted by edge count, ranks of 8
        # spread across cores, so PC[i] = max over near-equal counts.
        order = np.argsort(-wcnt, kind="stable")
        self.wmap = np.empty((n_cores, self.wpc), dtype=np.int64)
        for r in range(self.wpc):
            for k in range(n_cores):
                self.wmap[k, r] = order[r * n_cores + k]
        counts = wcnt[self.wmap]                  # [n_cores, wpc]
        self.PC = np.maximum(np.ceil(counts / P).astype(np.int64).max(axis=0), 1)
        self.T = int(self.PC.sum())

        self.slot_src = np.zeros((n_cores, self.T * P), dtype=np.int32)
        self.slot_dst = np.zeros((n_cores, self.T * P), dtype=np.int32)
        self.slot_rel = np.full((n_cores, self.T * P), int(PAD_REL), dtype=np.int32)
        for k in range(n_cores):
            t0 = 0
            for i in range(self.wpc):
                w = int(self.wmap[k, i])
                cnt = int(counts[k, i])
                if cnt > 0:
                    e0 = bounds[w]
                    sl = t0 * P
                    self.slot_src[k, sl:sl + cnt] = self.src_s[e0:e0 + cnt]
                    self.slot_dst[k, sl:sl + cnt] = self.dst_s[e0:e0 + cnt]
                    self.slot_rel[k, sl:sl + cnt] = self.dst_s[e0:e0 + cnt] - w * P
                t0 += int(self.PC[i])
        # rel as [P, T] f16 (col t = rel of edges t*P .. t*P+127)
        self.rel_pt = np.ascontiguousarray(
            self.slot_rel.reshape(n_cores, self.T, P).transpose(0, 2, 1)
        ).astype(np.float16)

    def build_stream(self, table, core, n_src_cols, n_dst_cols):
        """Pack per-edge gathered rows into the DMA-friendly stream layout
        [128][T, C] where C = n_src_cols + n_dst_cols.  The table is
        [n_win*P, Ctab] f16; cols [0:n_src_cols] are gathered by edge
        source, cols [n_src_cols:n_src_cols+n_dst_cols] by edge dest."""
        T, C = self.T, n_src_cols + n_dst_cols
        out = np.empty((P, T, C), dtype=np.float16)
        gs = table[self.slot_src[core], :n_src_cols]
        out[:, :, :n_src_cols] = gs.reshape(T, P, n_src_cols).transpose(1, 0, 2)
        gd = table[self.slot_dst[core], n_src_cols:n_src_cols + n_dst_cols]
        out[:, :, n_src_cols:n_src_cols + n_dst_cols] = (
            gd.reshape(T, P, n_dst_cols).transpose(1, 0, 2))
        return np.ascontiguousarray(out).reshape(P, T * C)

    def stream_S8(self, core):
        if not hasattr(self, "_s8"):
            self._s8 = {}
        if core not in self._s8:
            import ml_dtypes
            rel = self.rel_pt[core].astype(np.int32)        # [P, T]
            onehot = (rel[:, :, None] ==
                      np.arange(P, dtype=np.int32)[None, None, :])
            self._s8[core] = np.ascontiguousarray(
                onehot.astype(ml_dtypes.float8_e4m3)).reshape(P, self.T * P)
        return self._s8[core]

    def unshuffle(self, core_outs):
        """Reassemble per-core window-shuffled outputs into global node order."""
        C = core_outs[0].shape[1]
        out = np.empty((self.n_win * P, C), dtype=core_outs[0].dtype)
        ov = out.reshape(self.n_win, P, C)
        for k in range(self.C):
            ov[self.wmap[k]] = core_outs[k].reshape(self.wpc, P, C)
        return out


# ------------------------------------------------------------------ builders

def _build_node_kernel(wpc, c_in, c_out, bench_loop=1):
    """H = xT.T @ wext per node window; out [wpc*P, c_out] f16."""
    nc = bass.Bass()
    xT = nc.dram_tensor("xT", [c_in, wpc * P], F16, kind="ExternalInput")
    wext = nc.dram_tensor("wext", [c_in, c_out], F16, kind="ExternalInput")
    out = nc.dram_tensor("out", [wpc * P, c_out], F16, kind="ExternalOutput")

    NB = 3
    with tile.TileContext(nc) as tc:
        with (
            tc.tile_pool(name="const", bufs=1) as constp,
            tc.tile_pool(name="x", bufs=2) as xp,
            tc.tile_pool(name="o", bufs=3) as op_,
            tc.tile_pool(name="ps", bufs=2, space="PSUM") as psp,
        ):
            wext_sb = constp.tile([c_in, c_out], F16)
            nc.sync.dma_start(out=wext_sb[:], in_=wext[:])
            def node_phase(_iv=None):
                # whole node shard SBUF-resident; loaded inside the phase so
                # the bench loop counts the full x traffic
                xsh = xp.tile([c_in, wpc * P], F16, tag="xsh")
                nc.sync.dma_start(out=xsh[:], in_=xT[:])
                for c0 in range(0, wpc, NB):
                    nb = min(NB, wpc - c0)
                    ps = psp.tile([P, NB * c_out], F32, tag="ps")
                    for c in range(nb):
                        nc.tensor.matmul(
                            ps[:, c * c_out:(c + 1) * c_out],
                            xsh[:, (c0 + c) * P:(c0 + c + 1) * P], wext_sb[:],
                            start=True, stop=True)
                    ot = op_.tile([P, NB * c_out], F16, tag="ot")
                    nc.scalar.activation(ot[:, :nb * c_out], ps[:, :nb * c_out],
                                         AF.Copy)
                    dst = out[c0 * P:(c0 + nb) * P, :].rearrange(
                        "(c p) f -> p c f", p=P)
                    nc.sync.dma_start(
                        out=dst,
                        in_=ot[:, :nb * c_out].rearrange(
                            "p (c f) -> p c f", c=nb))

            if bench_loop > 1:
                with tc.For_i(0, bench_loop, 1) as _iv:
                    node_phase(_iv)
            else:
                node_phase()
    _finalize_kernel(nc)
    return nc


def _build_edge_kernel(T, PC, wpc, heads, hid, elu, add_bias, fuse_cols,
                       bench_loop=1):
    """Edge phase. Stream cols: [h (HC) | als (heads) | ald (heads)]; the
    one-hot S matrices arrive as a separate fp8 stream (exact for 0/1) and
    feed the PE directly as the segment-sum stationary (fp8 x f16 matmul).
    If fuse_cols > 0: epilogue computes o2 @ w2ext -> out [wpc*P, fuse_cols]
    f16 (layer-2 node phase fused in).  Else out is [wpc*P, HC] f32."""
    HC = heads * hid
    C = HC + 2 * heads
    nc = bass.Bass()
    xs = nc.dram_tensor("xs", [P, T * C], F16, kind="ExternalInput")
    s8 = nc.dram_tensor("s8", [P, T * P], F8, kind="ExternalInput")
    if fuse_cols:
        ident_c = nc.dram_tensor("ident", [P, P], F16, kind="ExternalInput")
        w2ext = nc.dram_tensor("w2ext", [P, fuse_cols], F16, kind="ExternalInput")
        out = nc.dram_tensor("out", [wpc * P, fuse_cols], F16,
                             kind="ExternalOutput")
    else:
        out = nc.dram_tensor("out", [wpc * P, HC], F32, kind="ExternalOutput")
    if add_bias:
        brep = nc.dram_tensor("brep", [P, HC], F32, kind="ExternalInput")

    GRP = CFG["grp"]
    n_groups = (T + GRP - 1) // GRP
    tile_win = []
    for i in range(wpc):
        tile_win += [i] * int(PC[i])
    first_of_win, last_of_win = {}, {}
    for t, w in enumerate(tile_win):
        first_of_win.setdefault(w, t)
        last_of_win[w] = t

    with tile.TileContext(nc) as tc:
        with (
            tc.tile_pool(name="const", bufs=1) as constp,
            tc.tile_pool(name="stream", bufs=3) as streamp,
            tc.tile_pool(name="smat", bufs=CFG["bufs"]) as sp_,
            tc.tile_pool(name="msg", bufs=CFG["bufs"]) as msgp,
            tc.tile_pool(name="zexp", bufs=CFG["bufs"]) as zp,
            tc.tile_pool(name="epi", bufs=2) as epip,
            tc.tile_pool(name="psW", bufs=CFG["psw_bufs"], space="PSUM") as psW,
            tc.tile_pool(name="psE", bufs=2, space="PSUM") as psE,
        ):
            if fuse_cols:
                ident_sb = constp.tile([P, P], F16)
                nc.sync.dma_start(out=ident_sb[:], in_=ident_c[:])
                w2_sb = constp.tile([P, fuse_cols], F16)
                nc.sync.dma_start(out=w2_sb[:], in_=w2ext[:])
            if add_bias:
                brep_sb = constp.tile([P, HC], F32)
                nc.sync.dma_start(out=brep_sb[:], in_=brep[:])
            ebias_sb = constp.tile([P, 1], F32)
            nc.vector.memset(ebias_sb[:], EXP_BIAS)

            def epilogue(w, psw):
                epi_eng = nc.gpsimd if CFG["epi_pool"] else nc.vector
                # every real node has a self-loop -> denom > 0; pad rows give
                # inf/NaN that are never gathered downstream
                rec = epip.tile([P, heads], F32, tag="rec")
                nc.vector.reciprocal(rec[:], psw[:, HC:HC + heads])
                o1 = epip.tile([P, HC], F32, tag="o1")
                r_ap = rec[:]
                nc.vector.tensor_tensor(
                    out=o1[:], in0=psw[:, 0:HC],
                    in1=bass.AP(r_ap.tensor, r_ap.offset,
                                [r_ap.ap[0], [1, heads], [0, hid]]),
                    op=OP.mult)
                if add_bias:
                    nc.vector.tensor_tensor(out=o1[:], in0=o1[:],
                                            in1=brep_sb[:], op=OP.add)
                if elu:
                    mn = epip.tile([P, HC], F32, tag="mn")
                    epi_eng.tensor_scalar(mn[:], o1[:], 0.0, None, OP.min)
                    ex = epip.tile([P, HC], F32, tag="ex")
                    nc.scalar.activation(ex[:], mn[:], AF.Exp)
                    mx = epip.tile([P, HC], F32, tag="mx")
                    epi_eng.tensor_scalar(mx[:], o1[:], 0.0, -1.0,
                                          OP.max, OP.add)
                else:
                    ex = mx = None
                if fuse_cols:
                    o2 = epip.tile([P, HC], F16, tag="o2")
                    if elu:
                        nc.vector.tensor_tensor(out=o2[:], in0=mx[:], in1=ex[:],
                                                op=OP.add)
                    else:
                        nc.vector.tensor_copy(o2[:], o1[:])
                    psT = psE.tile([P, P], F16, tag="psT")
                    nc.tensor.transpose(psT[:], o2[:], ident_sb[:])
                    o2T = epip.tile([P, P], F16, tag="o2T")
                    psH = psE.tile([P, fuse_cols], F32, tag="psH")
                    h2 = epip.tile([P, fuse_cols], F16, tag="h2")
                    if CFG["copy_act"]:
                        nc.scalar.activation(o2T[:], psT[:], AF.Copy)
                    else:
                        nc.vector.tensor_copy(o2T[:], psT[:])
                    nc.tensor.matmul(psH[:], o2T[:], w2_sb[:],
                                     start=True, stop=True)
                    if CFG["copy_act"]:
                        nc.scalar.activation(h2[:], psH[:], AF.Copy)
                    else:
                        nc.vector.tensor_copy(h2[:], psH[:])
                    nc.sync.dma_start(out=out[w * P:(w + 1) * P, :], in_=h2[:])
                else:
                    if elu:
                        res = epip.tile([P, HC], F32, tag="res")
                        nc.vector.tensor_tensor(out=res[:], in0=mx[:],
                                                in1=ex[:], op=OP.add)
                    else:
                        res = o1
                    nc.sync.dma_start(out=out[w * P:(w + 1) * P, :], in_=res[:])

            def edge_phase(_iv=None):
                psw_cur = None
                scount = [0]
                for g in range(n_groups):
                    tlo, thi = g * GRP, min(T, g * GRP + GRP)
                    ng = thi - tlo
                    xs_g = streamp.tile([P, GRP, C], F16, tag="xs")
                    nc.sync.dma_start(out=xs_g[:, :ng, :].rearrange(
                        "p g c -> p (g c)"),
                        in_=xs[:, tlo * C:thi * C])
                    S_g = sp_.tile([P, GRP, P], F8, tag="S")
                    nc.sync.dma_start(out=S_g[:, :ng, :].rearrange(
                        "p g c -> p (g c)"),
                        in_=s8[:, tlo * P:thi * P])
                    # z = als + ald  [P, ng, heads] f32
                    zf = zp.tile([P, GRP, heads], F32, tag="zf")
                    nc.vector.tensor_tensor(
                        out=zf[:, :ng, :], in0=xs_g[:, :ng, HC:HC + heads],
                        in1=xs_g[:, :ng, HC + heads:HC + 2 * heads], op=OP.add)
                    nc.scalar.activation(zf[:, :ng, :], zf[:, :ng, :],
                                         AF.Prelu, alpha=NEG_SLOPE)
                    # exp into the denominator columns of msg
                    msg_g = msgp.tile([P, GRP, HC + heads], F16, tag="msg")
                    nc.scalar.activation(msg_g[:, :ng, HC:HC + heads],
                                         zf[:, :ng, :], AF.Exp,
                                         bias=ebias_sb[:])
                    # messages = h * exp
                    ean, ead = CFG["exp_act_num"], CFG["exp_act_den"]
                    if (g % ead) < ean:
                        # ACT expands exp to all HC cols; DVE multiply is
                        # all-packed f16 (2x mode)
                        expf = zp.tile([P, GRP, HC], F16, tag="expf")
                        zin = zf[:, :ng, :]
                        zin_b = bass.AP(zin.tensor, zin.offset,
                                        [zin.ap[0], [heads, ng], [1, heads],
                                         [0, hid]])
                        nc.scalar.activation(expf[:, :ng, :], zin_b, AF.Exp,
                                             bias=ebias_sb[:])
                        nc.vector.tensor_tensor(
                            out=msg_g[:, :ng, 0:HC], in0=xs_g[:, :ng, 0:HC],
                            in1=expf[:, :ng, :], op=OP.mult)
                    else:
                        # DVE multiplies against broadcast exp cols (1x mode)
                        e_ap = msg_g[:, :ng, HC:HC + heads]
                        e_b = bass.AP(e_ap.tensor, e_ap.offset,
                                      [e_ap.ap[0], [HC + heads, ng],
                                       [1, heads], [0, hid]])
                        o_ap = msg_g[:, :ng, 0:HC]
                        o_b = bass.AP(o_ap.tensor, o_ap.offset,
                                      [o_ap.ap[0], [HC + heads, ng],
                                       [hid, heads], [1, hid]])
                        i_ap = xs_g[:, :ng, 0:HC]
                        i_b = bass.AP(i_ap.tensor, i_ap.offset,
                                      [i_ap.ap[0], [C, ng],
                                       [hid, heads], [1, hid]])
                        nc.vector.tensor_tensor(out=o_b, in0=i_b, in1=e_b,
                                                op=OP.mult)
                    for j in range(ng):
                        t = tlo + j
                        w = tile_win[t]
                        if t == first_of_win[w]:
                            psw_cur = psW.tile([P, HC + heads], F32, tag="psW")
                        nc.tensor.matmul(
                            psw_cur[:], S_g[:, j, :], msg_g[:, j, :],
                            start=(t == first_of_win[w]),
                            stop=(t == last_of_win[w]))
                        if t == last_of_win[w]:
                            epilogue(tile_win[t], psw_cur)

            if bench_loop > 1:
                with tc.For_i(0, bench_loop, 1) as _iv:
                    edge_phase(_iv)
            else:
                edge_phase()
    _finalize_kernel(nc)
    return nc


# ------------------------------------------------------------------ runner

def _fold_att(W, a):
    heads, hid = a.shape
    return np.einsum("ihc,hc->ih", W.reshape(W.shape[0], heads, hid), a)


class _GatRunner:
    def __init__(self, n_cores=N_CORES):
        self.C = n_cores
        self._graph = None
        self._graph_key = None
        self._kernels = {}

    def graph(self, edge_index, n_nodes):
        key = hash(np.asarray(edge_index).tobytes())
        if key != self._graph_key:
            self._graph = _Graph(edge_index, n_nodes, self.C)
            self._graph_key = key
            self._kernels.clear()
        return self._graph

    def node_kernel(self, g, c_in, c_out, bench_loop=1):
        key = ("N", g.T, c_in, c_out, bench_loop)
        if key not in self._kernels:
            self._kernels[key] = _build_node_kernel(g.wpc, c_in, c_out,
                                                    bench_loop)
        return self._kernels[key]

    def edge_kernel(self, name, g, heads, hid, elu, add_bias, fuse_cols,
                    bench_loop=1):
        key = (name, g.T, heads, hid, elu, add_bias, fuse_cols, bench_loop)
        if key not in self._kernels:
            old_grp = CFG["grp"]
            if name == "E2" and CFG.get("grp_e2"):
                CFG["grp"] = CFG["grp_e2"]
            try:
                self._kernels[key] = _build_edge_kernel(
                    g.T, g.PC, g.wpc, heads, hid, elu, add_bias, fuse_cols,
                    bench_loop)
            finally:
                CFG["grp"] = old_grp
        return self._kernels[key]

    @staticmethod
    def w1ext(W1, a_src1, a_dst1):
        return np.concatenate(
            [W1, _fold_att(W1, a_src1), _fold_att(W1, a_dst1)],
            axis=1).astype(np.float16)

    @staticmethod
    def w2ext(W2, a_src2, a_dst2):
        return np.concatenate(
            [W2, _fold_att(W2, a_src2), _fold_att(W2, a_dst2)],
            axis=1).astype(np.float16)

    def node_maps(self, g, x, wextv):
        xT_pad = np.zeros((x.shape[1], g.n_win * P), dtype=np.float16)
        xT_pad[:, :x.shape[0]] = np.asarray(x, np.float32).T
        return [{
            "xT": np.ascontiguousarray(
                xT_pad[:, k * g.shard_nodes:(k + 1) * g.shard_nodes]),
            "wext": wextv,
        } for k in range(self.C)]

    def edge_maps(self, g, table, heads, hid, fuse_w2=None, brep=None):
        HC = heads * hid
        maps = []
        for k in range(self.C):
            im = {
                "xs": g.build_stream(table, k, HC + heads, heads),
                "s8": g.stream_S8(k),
            }
            if fuse_w2 is not None:
                im["ident"] = np.eye(P, dtype=np.float16)
                im["w2ext"] = fuse_w2
            if brep is not None:
                im["brep"] = brep
            maps.append(im)
        return maps

    def run(self, x, edge_index, W1, a_src1, a_dst1, b1, W2, a_src2, a_dst2,
            b2):
        C = self.C
        N, IN_C = x.shape
        HEADS, HID = a_src1.shape
        HC = HEADS * HID
        OUT_C = W2.shape[1]
        g = self.graph(edge_index, N)
        b1nz = bool(np.any(b1))
        b2nz = bool(np.any(b2))
        assert not b1nz and not b2nz, "nonzero biases not wired up"

        w1e = self.w1ext(W1, a_src1, a_dst1)          # [IN_C, HC+2*HEADS]
        w2e = self.w2ext(W2, a_src2, a_dst2)          # [HC, OUT_C+2]

        ncN = self.node_kernel(g, IN_C, w1e.shape[1])
        resN = run_bass_kernel_spmd(ncN, self.node_maps(g, x, w1e),
                                    core_ids=list(range(C)))
        table1 = np.concatenate([r["out"] for r in resN.results], axis=0)

        ncE1 = self.edge_kernel("E1", g, HEADS, HID, True, False,
                                w2e.shape[1])
        mapsE1 = self.edge_maps(g, table1, HEADS, HID, fuse_w2=w2e)
        resE1 = run_bass_kernel_spmd(ncE1, mapsE1, core_ids=list(range(C)))
        table2 = g.unshuffle([r["out"] for r in resE1.results])

        ncE2 = self.edge_kernel("E2", g, 1, OUT_C, False, False, 0)
        mapsE2 = self.edge_maps(g, table2, 1, OUT_C)
        resE2 = run_bass_kernel_spmd(ncE2, mapsE2, core_ids=list(range(C)))
        return g.unshuffle([r["out"] for r in resE2.results])[:N]


_RUNNER = _GatRunner()


def kernel(x, edge_index, W1, a_src1, a_dst1, b1, W2, a_src2, a_dst2, b2):
    """Full-input / full-output entry point. Returns [N, OUT_C] float32."""
    args = [np.asarray(v) for v in
            (x, edge_index, W1, a_src1, a_dst1, b1, W2, a_src2, a_dst2, b2)]
    return _RUNNER.run(*args).astype(np.float32)
